# revision 16
# baseline (speedup 1.0000x reference)
"""BatchHardTripletLoss on 8 Trainium2 NeuronCores.

Math (per sorted-by-label data):
  e = embeddings / ||embeddings||          (row L2 norm)
  S = e @ e.T                              (cosine similarity Gram matrix)
  T = S - 4 * [label_i == label_j]
  loss_row = relu(max_j T - min_j T - 3.7)  (= relu(hard_pos - hard_neg + 0.3))
  out = mean(loss_row)

min_j T always lands on a same-label element (penalty -4 beats any s >= -1);
self (s=1) is never the min unless the row has no other positive, in which
case max_j T < 0.7 - 1e9... (verified: global max non-same s = 0.304) makes
the relu zero either way.

Sharding: rows sorted by label, grouped into 64 tiles of 128 rows. Core c
owns global row-tiles g = 8m + c (m = 0..7, interleaved). With sorted labels,
all positives of row-tile g live in columns [128g - Cmax, 128g + 128 + Cmax);
for every core the m-th tile's positive window is inside the *same* column
window W(m) = [1024m - 128, 1024m + 1280), so one SPMD program works for all
cores: the eq-label mask + min-mining runs only on W(m), plain max-mining on
the rest. Requires max label multiplicity <= 129 (checked at runtime).
"""

import numpy as np
from contextlib import ExitStack

N, D = 8192, 512
NCORES = 8
M_TILES = 8          # row tiles per core
K_TILES = D // 128   # 4
NQ = 4               # column quads of 2048
QW = 2048
MARGIN_C = 3.7       # 4 - 1 + MARGIN(0.3); loss = relu(maxT - minT - 3.7)


def _window(m):
    """Column window [lo, hi) that contains every positive of row-tile m on
    every core (global tiles g = 8m + c, c in 0..7)."""
    lo = max(0, 1024 * m - 128)
    hi = min(N, 1024 * m + 1024 + 256)
    return lo, hi


def _build_program():
    import concourse.bass as bass
    import concourse.bacc as bacc
    import concourse.tile as tile
    from concourse import mybir

    f16 = mybir.dt.float16
    f32 = mybir.dt.float32
    Alu = mybir.AluOpType
    Act = mybir.ActivationFunctionType
    Ax = mybir.AxisListType

    nc = bacc.Bacc("TRN2", target_bir_lowering=False, debug=False,
                   num_devices=NCORES)

    emb = nc.dram_tensor("emb", [N, D], f16, kind="ExternalInput").ap()
    blk = nc.dram_tensor("blk", [128 * M_TILES, D], f16,
                         kind="ExternalInput").ap()
    labs = nc.dram_tensor("labs", [N], f16, kind="ExternalInput").ap()
    blklab = nc.dram_tensor("blklab", [128 * M_TILES], f32,
                            kind="ExternalInput").ap()
    out = nc.dram_tensor("out", [1, 1], f32, kind="ExternalOutput").ap()

    NEG = -1.0e30
    POS = 1.0e30

    with TileCtx(nc, tile) as (tc, ctx):
        # ---------------- pools ----------------
        persist = ctx.enter_context(tc.tile_pool(name="persist", bufs=1))
        natp = ctx.enter_context(tc.tile_pool(name="nat", bufs=1))
        psum = ctx.enter_context(tc.tile_pool(name="ps", bufs=2, space="PSUM"))
        eqp = ctx.enter_context(tc.tile_pool(name="eq", bufs=2))
        twp = ctx.enter_context(tc.tile_pool(name="tw", bufs=2))

        # persistent SBUF tensors
        labels_sb = persist.tile([128, N], f16, tag="labels")
        blklab_sb = persist.tile([128, M_TILES], f32, tag="blklab")
        # transposed normalized embeddings: 16 column tiles of 512, each
        # holding the 4 K-chunks side by side: ENT[j][p, k*512+i] =
        # e_norm[j*512+i, k*128+p]
        ENT = [persist.tile([128, K_TILES * 512], f16, tag=f"ent{j}",
                            name=f"ent{j}")
               for j in range(16)]
        BlkT = persist.tile([128, K_TILES * 1024], f16, tag="blkt")
        ss_blk = persist.tile([128, M_TILES], f32, tag="ssblk")
        r_blk = persist.tile([128, M_TILES], f32, tag="rblk")
        ss_all = persist.tile([128, 64], f32, tag="ssall")
        r_all = persist.tile([128, 64], f32, tag="rall")
        maxp = persist.tile([128, M_TILES * 6], f32, tag="maxp")
        minp = persist.tile([128, M_TILES * 2], f32, tag="minp")
        maxT = persist.tile([128, M_TILES], f32, tag="maxT")
        minT = persist.tile([128, M_TILES], f32, tag="minT")
        diffs = persist.tile([128, M_TILES], f32, tag="diffs")
        relu_d = persist.tile([128, M_TILES], f32, tag="relud")
        row_loss = persist.tile([128, 1], f32, tag="rowloss")
        ones_sb = persist.tile([128, 1], f32, tag="ones")
        negm = persist.tile([128, 1], f32, tag="negm")
        out_sb = persist.tile([1, 1], f32, tag="outsb")
        sqdump = persist.tile([128, D], f16, tag="sqdump")

        nc.vector.memset(maxp[:], NEG)
        nc.vector.memset(minp[:], POS)
        nc.vector.memset(ones_sb[:], 1.0)
        nc.vector.memset(negm[:], -MARGIN_C)

        # ---------------- label loads ----------------
        labs_b = bass.AP(labs.tensor, labs.offset, [[0, 128], [1, N]])
        nc.sync.dma_start(out=labels_sb[:], in_=labs_b)
        nc.sync.dma_start(out=blklab_sb[:],
                          in_=blklab.rearrange("(m p) -> p m", p=128))

        # ---------------- block: normalize + transpose ----------------
        blk_nat = [natp.tile([128, D], f16, tag=f"bnat{t}", name=f"bnat{t}")
                   for t in range(M_TILES)]
        for t in range(M_TILES):
            nc.sync.dma_start(out=blk_nat[t][:],
                              in_=blk[t * 128:(t + 1) * 128, :])
            nc.scalar.activation(sqdump[:], blk_nat[t][:], Act.Square,
                                 accum_out=ss_blk[:, t:t + 1])
        nc.scalar.activation(r_blk[:], ss_blk[:], Act.Sqrt)
        nc.vector.reciprocal(r_blk[:], r_blk[:])
        for t in range(M_TILES):
            nc.scalar.activation(blk_nat[t][:], blk_nat[t][:], Act.Copy,
                                 scale=r_blk[:, t:t + 1])
            for k in range(K_TILES):
                nc.sync.dma_start_transpose(
                    out=BlkT[:, k * 1024 + t * 128:k * 1024 + (t + 1) * 128],
                    in_=blk_nat[t][:, k * 128:(k + 1) * 128])

        # ---------------- full matrix: normalize + transpose, 4 groups ----
        for grp in range(4):
            ts0 = 16 * grp
            nats = [natp.tile([128, D], f16, tag=f"nat{t % 16}",
                              name=f"nat{grp}_{t % 16}")
                    for t in range(ts0, ts0 + 16)]
            for i, t in enumerate(range(ts0, ts0 + 16)):
                nc.sync.dma_start(out=nats[i][:],
                                  in_=emb[t * 128:(t + 1) * 128, :])
                nc.scalar.activation(sqdump[:], nats[i][:], Act.Square,
                                     accum_out=ss_all[:, t:t + 1])
            nc.scalar.activation(r_all[:, ts0:ts0 + 16],
                                 ss_all[:, ts0:ts0 + 16], Act.Sqrt)
            nc.vector.reciprocal(r_all[:, ts0:ts0 + 16],
                                 r_all[:, ts0:ts0 + 16])
            for i, t in enumerate(range(ts0, ts0 + 16)):
                nc.scalar.activation(nats[i][:], nats[i][:], Act.Copy,
                                     scale=r_all[:, t:t + 1])
                j = t // 4
                off = (t % 4) * 128
                for k in range(K_TILES):
                    nc.sync.dma_start_transpose(
                        out=ENT[j][:, k * 512 + off:k * 512 + off + 128],
                        in_=nats[i][:, k * 128:(k + 1) * 128])

        # ---------------- mining ----------------
        # per (m): pieces of each quad = clean segments + window segments.
        # Slot ids for the maxp/minp partials are assigned globally per m
        # across all quads (they all land in m's slot range).
        piece_table = {}
        for m in range(M_TILES):
            wlo, whi = _window(m)
            nslot = 0
            wslot = 0
            for q in range(NQ):
                qlo, qhi = q * QW, (q + 1) * QW
                a, b = max(qlo, wlo), min(qhi, whi)
                pieces = []      # (lo, hi, is_window, slot)
                if a >= b:
                    pieces.append((qlo, qhi, False, nslot))
                    nslot += 1
                else:
                    if qlo < a:
                        pieces.append((qlo, a, False, nslot))
                        nslot += 1
                    pieces.append((a, b, True, (nslot, wslot)))
                    nslot += 1
                    wslot += 1
                    if b < qhi:
                        pieces.append((b, qhi, False, nslot))
                        nslot += 1
                piece_table[(q, m)] = pieces
            assert nslot <= 6 and wslot <= 2, (m, nslot, wslot)

        for q in range(NQ):
            for m in range(M_TILES):
                ps = psum.tile([128, QW], f32, tag="ps")
                for k in range(K_TILES):
                    lhsT = BlkT[:, k * 1024 + m * 128:k * 1024 + (m + 1) * 128]
                    for j in range(4):
                        n = q * 4 + j
                        nc.tensor.matmul(
                            ps[:, j * 512:(j + 1) * 512],
                            lhsT=lhsT,
                            rhs=ENT[n][:, k * 512:(k + 1) * 512],
                            start=(k == 0), stop=(k == K_TILES - 1))

                qlo = q * QW
                for (lo, hi, isw, slot) in piece_table[(q, m)]:
                    w = hi - lo
                    pslice = ps[:, lo - qlo:hi - qlo]
                    if not isw:
                        nc.vector.tensor_reduce(
                            out=maxp[:, m * 6 + slot:m * 6 + slot + 1],
                            in_=pslice, axis=Ax.X, op=Alu.max)
                    else:
                        nslot, wslot = slot
                        eq4 = eqp.tile([128, 1280], f32, tag="eq4")
                        nc.vector.tensor_scalar(
                            out=eq4[:, :w], in0=labels_sb[:, lo:hi],
                            scalar1=blklab_sb[:, m:m + 1], scalar2=4.0,
                            op0=Alu.is_equal, op1=Alu.mult)
                        tw = twp.tile([128, 1280], f32, tag="tw")
                        nc.vector.tensor_tensor(
                            out=tw[:, :w], in0=pslice, in1=eq4[:, :w],
                            op=Alu.subtract)
                        nc.vector.tensor_reduce(
                            out=maxp[:, m * 6 + nslot:m * 6 + nslot + 1],
                            in_=tw[:, :w], axis=Ax.X, op=Alu.max)
                        nc.vector.tensor_reduce(
                            out=minp[:, m * 2 + wslot:m * 2 + wslot + 1],
                            in_=tw[:, :w], axis=Ax.X, op=Alu.min)

        # ---------------- finale ----------------
        for m in range(M_TILES):
            nc.vector.tensor_reduce(out=maxT[:, m:m + 1],
                                    in_=maxp[:, m * 6:(m + 1) * 6],
                                    axis=Ax.X, op=Alu.max)
            nc.vector.tensor_reduce(out=minT[:, m:m + 1],
                                    in_=minp[:, m * 2:(m + 1) * 2],
                                    axis=Ax.X, op=Alu.min)
        nc.vector.tensor_tensor(out=diffs[:], in0=maxT[:], in1=minT[:],
                                op=Alu.subtract)
        nc.scalar.activation(relu_d[:], diffs[:], Act.Relu, bias=negm[:],
                             accum_out=row_loss[:])
        ps1 = psum.tile([1, 1], f32, tag="ps")
        nc.tensor.matmul(ps1[:], lhsT=row_loss[:], rhs=ones_sb[:],
                         start=True, stop=True)
        nc.scalar.copy(out_sb[:], ps1[:])
        nc.sync.dma_start(out=out, in_=out_sb[:])

    nc.compile()
    return nc


class TileCtx:
    """contextmanager pairing TileContext with an ExitStack."""

    def __init__(self, nc, tile_mod):
        self.nc = nc
        self.tile_mod = tile_mod

    def __enter__(self):
        self.ctx = ExitStack()
        self.ctx.__enter__()
        self.tc = self.tile_mod.TileContext(self.nc)
        self.tc.__enter__()
        return self.tc, self.ctx

    def __exit__(self, *exc):
        self.ctx.__exit__(*exc)
        return self.tc.__exit__(*exc)


def kernel(embeddings, labels):
    from concourse.bass_utils import run_bass_kernel_spmd

    E = np.ascontiguousarray(np.asarray(embeddings, dtype=np.float32))
    lab = np.asarray(labels).reshape(-1)
    assert E.shape == (N, D)

    # sort rows by label; mean loss is permutation invariant
    order = np.argsort(lab, kind="stable")
    E_s = E[order]
    lab_s = lab[order].astype(np.int64)
    assert np.bincount(lab_s).max() <= 129, "label multiplicity > 129"

    E16 = E_s.astype(np.float16)
    lab16 = lab_s.astype(np.float16)

    # interleaved sharding: core c owns global row-tiles g = 8m + c
    tiles = E16.reshape(64, 128, D)
    labt = lab16.reshape(64, 128)
    in_maps = []
    for c in range(NCORES):
        gsel = [8 * m + c for m in range(M_TILES)]
        in_maps.append({
            "emb": E16,
            "blk": np.ascontiguousarray(
                tiles[gsel].reshape(128 * M_TILES, D)),
            "labs": lab16,
            "blklab": np.ascontiguousarray(
                labt[gsel].reshape(-1).astype(np.float32)),
        })

    nc = _build_program()
    res = run_bass_kernel_spmd(nc, in_maps, core_ids=list(range(NCORES)))
    global LAST_RESULTS
    LAST_RESULTS = res
    total = sum(float(r["out"][0, 0]) for r in res.results)
    return np.float32(total / N)


LAST_RESULTS = None


# revision 18
# speedup vs baseline: 2.0255x; 2.0255x over previous
"""BatchHardTripletLoss on 8 Trainium2 NeuronCores.

Math (on rows sorted by label):
  e = embeddings / ||embeddings||          (row L2 norm)
  S = e @ e.T                              (cosine similarity Gram matrix)
  T = S - 4 * [label_i == label_j]
  loss_row = relu(max_j T - min_j T - 3.7)  (= relu(hard_pos - hard_neg + 0.3))
  out = mean(loss_row)

min_j T always lands on a same-label element (the -4 shift beats any s >= -1);
self (s=1) is never the min unless the row has no other positive, in which
case max_j T < 0.7 keeps the relu at zero either way (verified: global max
non-same s = 0.304 for this input family).

Sharding: rows sorted by label, grouped into 64 tiles of 128 rows. Core c
owns global row-tiles g = 8m + c (m = 0..7, interleaved). With sorted labels,
all positives of row-tile g live in columns [128g - Cmax, 128g + 128 + Cmax);
for every core the m-th tile's positive window is inside the *same* column
window W(m) = [1024m - 128, 1024m + 1280), so one SPMD program serves all
cores: the eq-label mask + min-mining runs only on W(m), plain max mining on
the rest. Requires max label multiplicity <= 129 (checked at runtime).

Layout: the host ships the embeddings both natural ([N, D], for row norms)
and transposed ([D, N], the matmul operand). The device computes
r = 1/||row|| in natural layout, round-trips r through DRAM to get it
replicated across partitions, and column-scales the transposed operand
in place on GpSimd. No on-device transposes (the DMA xbar transpose
serializes on the Sync engine at ~1.2 us per 128x128 chunk).
"""

import numpy as np
from contextlib import ExitStack

N, D = 8192, 512
NCORES = 8
M_TILES = 8          # row tiles per core
K_TILES = D // 128   # 4
NQ = 4               # column quads of 2048
QW = 2048
MARGIN_C = 3.7       # 4 - 1 + MARGIN(0.3); loss = relu(maxT - minT - 3.7)


def _window(m):
    """Column window [lo, hi) containing every positive of row-tile m on
    every core (global tiles g = 8m + c, c in 0..7)."""
    lo = max(0, 1024 * m - 128)
    hi = min(N, 1024 * m + 1024 + 256)
    return lo, hi


def _pieces():
    """piece_table[(q, m)] = [(lo, hi, is_window, slot)] with slot ids
    assigned globally per m across quads."""
    table = {}
    for m in range(M_TILES):
        wlo, whi = _window(m)
        nslot = 0
        wslot = 0
        for q in range(NQ):
            qlo, qhi = q * QW, (q + 1) * QW
            a, b = max(qlo, wlo), min(qhi, whi)
            pieces = []
            if a >= b:
                pieces.append((qlo, qhi, False, nslot))
                nslot += 1
            else:
                if qlo < a:
                    pieces.append((qlo, a, False, nslot))
                    nslot += 1
                pieces.append((a, b, True, (nslot, wslot)))
                nslot += 1
                wslot += 1
                if b < qhi:
                    pieces.append((b, qhi, False, nslot))
                    nslot += 1
            table[(q, m)] = pieces
        assert nslot <= 6 and wslot <= 2, (m, nslot, wslot)
    return table


def _build_program():
    import concourse.bass as bass
    import concourse.bacc as bacc
    import concourse.tile as tile
    from concourse import mybir

    f16 = mybir.dt.float16
    f32 = mybir.dt.float32
    Alu = mybir.AluOpType
    Act = mybir.ActivationFunctionType
    Ax = mybir.AxisListType

    nc = bacc.Bacc("TRN2", target_bir_lowering=False, debug=False,
                   num_devices=NCORES)

    embT = nc.dram_tensor("embT", [D, N], f16, kind="ExternalInput").ap()
    emb = nc.dram_tensor("emb", [N, D], f16, kind="ExternalInput").ap()
    blkT = nc.dram_tensor("blkT", [128, K_TILES * 1024], f16,
                          kind="ExternalInput").ap()
    blkn = nc.dram_tensor("blkn", [128 * M_TILES, D], f16,
                          kind="ExternalInput").ap()
    labs = nc.dram_tensor("labs", [N], f16, kind="ExternalInput").ap()
    blklab = nc.dram_tensor("blklab", [128 * M_TILES], f32,
                            kind="ExternalInput").ap()
    out = nc.dram_tensor("out", [1, 1], f32, kind="ExternalOutput").ap()
    # DRAM scratch for the norm round-trip
    rall_d = nc.dram_tensor("rall_d", [N], f32).ap()
    rblk_d = nc.dram_tensor("rblk_d", [128 * M_TILES], f32).ap()

    NEG = -1.0e30
    POS = 1.0e30
    ptab = _pieces()

    with TileCtx(nc, tile) as (tc, ctx):
        persist = ctx.enter_context(tc.tile_pool(name="persist", bufs=1))
        natp = ctx.enter_context(tc.tile_pool(name="nat", bufs=1))
        psum = ctx.enter_context(tc.tile_pool(name="ps", bufs=2, space="PSUM"))
        eqp = ctx.enter_context(tc.tile_pool(name="eq", bufs=2))
        twp = ctx.enter_context(tc.tile_pool(name="tw", bufs=2))

        labels_sb = persist.tile([128, N], f16, tag="labels")
        blklab_sb = persist.tile([128, M_TILES], f32, tag="blklab")
        # ET[k][g]: [128, 2048] fp16 — embT rows k*128..(k+1)*128, col group g
        ET = [[persist.tile([128, QW], f16, tag=f"et{k}_{g}",
                            name=f"et{k}_{g}") for g in range(NQ)]
              for k in range(K_TILES)]
        BlkT = persist.tile([128, K_TILES * 1024], f16, tag="blkt")
        Rg = [persist.tile([128, QW], f32, tag=f"rg{g}", name=f"rg{g}")
              for g in range(NQ)]
        Rblk = persist.tile([128, 1024], f32, tag="rblk2")
        ss_blk = persist.tile([128, M_TILES], f32, tag="ssblk")
        r_blk = persist.tile([128, M_TILES], f32, tag="rblk")
        ss_all = persist.tile([128, 64], f32, tag="ssall")
        r_all = persist.tile([128, 64], f32, tag="rall")
        maxp = persist.tile([128, M_TILES * 6], f32, tag="maxp")
        minp = persist.tile([128, M_TILES * 2], f32, tag="minp")
        maxT = persist.tile([128, M_TILES], f32, tag="maxT")
        minT = persist.tile([128, M_TILES], f32, tag="minT")
        diffs = persist.tile([128, M_TILES], f32, tag="diffs")
        relu_d = persist.tile([128, M_TILES], f32, tag="relud")
        row_loss = persist.tile([128, 1], f32, tag="rowloss")
        ones_sb = persist.tile([128, 1], f32, tag="ones")
        negm = persist.tile([128, 1], f32, tag="negm")
        out_sb = persist.tile([1, 1], f32, tag="outsb")
        sqdump = persist.tile([128, D], f16, tag="sqdump")

        nc.vector.memset(maxp[:], NEG)
        nc.vector.memset(minp[:], POS)
        nc.vector.memset(ones_sb[:], 1.0)
        nc.vector.memset(negm[:], -MARGIN_C)

        # ---------------- labels ----------------
        labs_b = bass.AP(labs.tensor, labs.offset, [[0, 128], [1, N]])
        nc.sync.dma_start(out=labels_sb[:], in_=labs_b)
        nc.sync.dma_start(out=blklab_sb[:],
                          in_=blklab.rearrange("(m p) -> p m", p=128))

        # ---------------- block: norms + scale ----------------
        nc.sync.dma_start(out=BlkT[:], in_=blkT)
        for t in range(M_TILES):
            bn = natp.tile([128, D], f16, tag=f"bnat{t}", name=f"bnat{t}")
            nc.sync.dma_start(out=bn[:], in_=blkn[t * 128:(t + 1) * 128, :])
            nc.scalar.activation(sqdump[:], bn[:], Act.Square,
                                 accum_out=ss_blk[:, t:t + 1])
        nc.scalar.activation(r_blk[:], ss_blk[:], Act.Sqrt)
        nc.vector.reciprocal(r_blk[:], r_blk[:])
        nc.sync.dma_start(out=rblk_d.rearrange("(t p) -> p t", p=128),
                          in_=r_blk[:])
        rblk_b = bass.AP(rblk_d.tensor, rblk_d.offset, [[0, 128], [1, 1024]])
        nc.sync.dma_start(out=Rblk[:], in_=rblk_b)
        for k in range(K_TILES):
            nc.gpsimd.tensor_tensor(
                out=BlkT[:, k * 1024:(k + 1) * 1024],
                in0=BlkT[:, k * 1024:(k + 1) * 1024],
                in1=Rblk[:], op=Alu.mult)

        # ---------------- full matrix: per column-group norm + scale ------
        for k in range(K_TILES):
            for g in range(NQ):
                nc.sync.dma_start(
                    out=ET[k][g][:],
                    in_=embT[k * 128:(k + 1) * 128, g * QW:(g + 1) * QW])
        for g in range(NQ):
            ts0 = 16 * g
            for t in range(ts0, ts0 + 16):
                nt = natp.tile([128, D], f16, tag=f"nat{t % 16}",
                               name=f"nat{g}_{t % 16}")
                nc.sync.dma_start(out=nt[:],
                                  in_=emb[t * 128:(t + 1) * 128, :])
                nc.scalar.activation(sqdump[:], nt[:], Act.Square,
                                     accum_out=ss_all[:, t:t + 1])
            nc.scalar.activation(r_all[:, ts0:ts0 + 16],
                                 ss_all[:, ts0:ts0 + 16], Act.Sqrt)
            nc.vector.reciprocal(r_all[:, ts0:ts0 + 16],
                                 r_all[:, ts0:ts0 + 16])
            seg = rall_d[g * QW:(g + 1) * QW]
            nc.sync.dma_start(out=seg.rearrange("(t p) -> p t", p=128),
                              in_=r_all[:, ts0:ts0 + 16])
            rall_b = bass.AP(seg.tensor, seg.offset, [[0, 128], [1, QW]])
            nc.sync.dma_start(out=Rg[g][:], in_=rall_b)
            for k in range(K_TILES):
                nc.gpsimd.tensor_tensor(out=ET[k][g][:], in0=ET[k][g][:],
                                        in1=Rg[g][:], op=Alu.mult)

        # ---------------- mining ----------------
        for q in range(NQ):
            for m in range(M_TILES):
                ps = psum.tile([128, QW], f32, tag="ps")
                for k in range(K_TILES):
                    lhsT = BlkT[:, k * 1024 + m * 128:k * 1024 + (m + 1) * 128]
                    for j in range(4):
                        nc.tensor.matmul(
                            ps[:, j * 512:(j + 1) * 512],
                            lhsT=lhsT,
                            rhs=ET[k][q][:, j * 512:(j + 1) * 512],
                            start=(k == 0), stop=(k == K_TILES - 1))

                qlo = q * QW
                for (lo, hi, isw, slot) in ptab[(q, m)]:
                    w = hi - lo
                    pslice = ps[:, lo - qlo:hi - qlo]
                    if not isw:
                        nc.vector.tensor_reduce(
                            out=maxp[:, m * 6 + slot:m * 6 + slot + 1],
                            in_=pslice, axis=Ax.X, op=Alu.max)
                    else:
                        nslot, wslot = slot
                        eq4 = eqp.tile([128, 1280], f32, tag="eq4")
                        nc.vector.tensor_scalar(
                            out=eq4[:, :w], in0=labels_sb[:, lo:hi],
                            scalar1=blklab_sb[:, m:m + 1], scalar2=4.0,
                            op0=Alu.is_equal, op1=Alu.mult)
                        tw = twp.tile([128, 1280], f32, tag="tw")
                        nc.vector.tensor_tensor(
                            out=tw[:, :w], in0=pslice, in1=eq4[:, :w],
                            op=Alu.subtract)
                        nc.vector.tensor_reduce(
                            out=maxp[:, m * 6 + nslot:m * 6 + nslot + 1],
                            in_=tw[:, :w], axis=Ax.X, op=Alu.max)
                        nc.vector.tensor_reduce(
                            out=minp[:, m * 2 + wslot:m * 2 + wslot + 1],
                            in_=tw[:, :w], axis=Ax.X, op=Alu.min)

        # ---------------- finale ----------------
        for m in range(M_TILES):
            nc.vector.tensor_reduce(out=maxT[:, m:m + 1],
                                    in_=maxp[:, m * 6:(m + 1) * 6],
                                    axis=Ax.X, op=Alu.max)
            nc.vector.tensor_reduce(out=minT[:, m:m + 1],
                                    in_=minp[:, m * 2:(m + 1) * 2],
                                    axis=Ax.X, op=Alu.min)
        nc.vector.tensor_tensor(out=diffs[:], in0=maxT[:], in1=minT[:],
                                op=Alu.subtract)
        nc.scalar.activation(relu_d[:], diffs[:], Act.Relu, bias=negm[:],
                             accum_out=row_loss[:])
        ps1 = psum.tile([1, 1], f32, tag="ps")
        nc.tensor.matmul(ps1[:], lhsT=row_loss[:], rhs=ones_sb[:],
                         start=True, stop=True)
        nc.scalar.copy(out_sb[:], ps1[:])
        nc.sync.dma_start(out=out, in_=out_sb[:])

    nc.compile()
    return nc


class TileCtx:
    """contextmanager pairing TileContext with an ExitStack (pools close
    before the TileContext schedules)."""

    def __init__(self, nc, tile_mod):
        self.nc = nc
        self.tile_mod = tile_mod

    def __enter__(self):
        self.ctx = ExitStack()
        self.ctx.__enter__()
        self.tc = self.tile_mod.TileContext(self.nc)
        self.tc.__enter__()
        return self.tc, self.ctx

    def __exit__(self, *exc):
        self.ctx.__exit__(*exc)
        return self.tc.__exit__(*exc)


def _prep_inputs(embeddings, labels):
    E = np.ascontiguousarray(np.asarray(embeddings, dtype=np.float32))
    lab = np.asarray(labels).reshape(-1)
    assert E.shape == (N, D)

    order = np.argsort(lab, kind="stable")
    E_s = E[order]
    lab_s = lab[order].astype(np.int64)
    assert np.bincount(lab_s).max() <= 129, "label multiplicity > 129"

    E16 = E_s.astype(np.float16)
    lab16 = lab_s.astype(np.float16)
    embT16 = np.ascontiguousarray(E16.T)

    tiles = E16.reshape(64, 128, D)
    labt = lab16.reshape(64, 128)
    in_maps = []
    for c in range(NCORES):
        gsel = [8 * m + c for m in range(M_TILES)]
        blk = np.ascontiguousarray(tiles[gsel].reshape(128 * M_TILES, D))
        # blkT[p, k*1024 + j] = blk[j, k*128 + p]
        blkT = np.ascontiguousarray(
            blk.reshape(1024, K_TILES, 128).transpose(2, 1, 0)
            .reshape(128, K_TILES * 1024))
        in_maps.append({
            "embT": embT16,
            "emb": E16,
            "blkT": blkT,
            "blkn": blk,
            "labs": lab16,
            "blklab": np.ascontiguousarray(
                labt[gsel].reshape(-1).astype(np.float32)),
        })
    return in_maps


def kernel(embeddings, labels):
    from concourse.bass_utils import run_bass_kernel_spmd

    in_maps = _prep_inputs(embeddings, labels)
    nc = _build_program()
    res = run_bass_kernel_spmd(nc, in_maps, core_ids=list(range(NCORES)))
    global LAST_RESULTS
    LAST_RESULTS = res
    total = sum(float(r["out"][0, 0]) for r in res.results)
    return np.float32(total / N)


LAST_RESULTS = None
